# revision 12
# baseline (speedup 1.0000x reference)
"""Trainium2 Bass kernel: low-rank (LoRA-style) linear with 2:4 soft-threshold
pruned weights, fp16 matmul / fp32 accumulate.

  wA = soft_threshold24(weight_A) * scale_A          # [IN, R]
  wB = soft_threshold24(weight_B) * scale_B          # [OUT, R]
  x_proj = f16(x) @ f16(wA)            (f32 accum)   # [N, R]
  out    = f16(x_proj) @ f16(wB).T + bias            # [N, OUT]

Sharding: data-parallel over the token dim across 8 cores (2048 tokens/core),
small weights replicated. No collectives.

v2 layout: the host uploads each core's x shard already transposed
([IN_F, T_CORE] f16), so mm1 is a plain f16 matmul (wA tile stationary,
xT tile moving) with NO on-device transposes of x -- the v1 kernel spent
half its PE time on f32 pair-transposes + PSUM drain copies. PE work drops
to 256 matmuls of 512 cols; the kernel becomes DMA-bound (~33.5 MB at
~360 GB/s => ~96 us floor).

Schedule: 32 xT k-tiles [128, 2048] stream in on the SP queue (0.5 MB
contiguous each). mm1 runs k-outer / token-block-inner, accumulating
x_proj[64, 2048] across 4 PSUM banks, so it consumes tiles as they land.
wA's threshold runs on DVE (needed by mm1 k=0 early); wB's runs on the
Pool engine concurrently (only needed ~45 us in). The 32 wB.T PE
transposes hide in mm1's tail slack (k>=24, load-paced). mm2 then streams
16 token tiles x 8 out-blocks vs the wbt [65, 4096] stationary-side tiles
(ones row + bias row fold the bias into the matmul), drains PSUM on
alternating ACT/DVE, and stores ride the now-idle SP queue.
"""

import sys

import numpy as np

if "/opt/trn_rl_repo" not in sys.path:
    sys.path.insert(0, "/opt/trn_rl_repo")

B, S, IN_F, OUT_F, RANK = 4, 4096, 4096, 4096, 64
N_CORES = 8
N_TOK = B * S                   # 16384
T_CORE = N_TOK // N_CORES       # 2048 tokens per core
P = 128
N_K = IN_F // P                 # 32 contraction tiles
TB = 4                          # token blocks for mm1 PSUM banks
TBLK = T_CORE // TB             # 512
MM2_N = 512
N_OB = OUT_F // MM2_N           # 8 output column groups
N_TT = T_CORE // P              # 16 token tiles for mm2

_CACHE = {}


def _soft_threshold_weights(nc, pool, w_dram, scale, out_f16, eng, pfx):
    """Emit IR computing soft_threshold24(w_dram f16) * scale (f16) on `eng`.

    w_dram: [IN_or_OUT, RANK] f16, already host-permuted so that partition
    p's nb rows are contiguous: a plain 4KB-descriptor load lands
    row-permuted weights as [p, n, r]. The 2:4 threshold is elementwise
    over groups of 4 along R, so any row permutation works.
    """
    import concourse.mybir as mybir

    f16 = mybir.dt.float16
    nb = w_dram.shape[0] // P
    wf = pool.tile([P, nb, RANK], f16, tag=pfx + "wstage", name="wstage")
    nc.scalar.dma_start(wf[:], w_dram[:].rearrange("(c n) r -> c n r", c=P))

    thr = pool.tile([P, nb, RANK], f16, tag=pfx + "wthr", name="wthr")
    amin = mybir.AluOpType.min
    amx = mybir.AluOpType.max
    ve = eng

    wfh = wf[:]
    g = wfh.rearrange("p b (g q) -> p b g q", q=4)
    gj = [g[:, :, :, j : j + 1] for j in range(4)]
    ash = [P, nb, RANK // 4, 1]
    wneg = pool.tile([P, nb, RANK], f16, tag=pfx + "wneg", name="wneg")
    ve.tensor_scalar_mul(wneg[:], wfh, -1.0)
    ng = wneg[:].rearrange("p b (g q) -> p b g q", q=4)
    ab = [pool.tile(ash, f16, tag=f"{pfx}abs{j}", name=f"abs{j}")
          for j in range(4)]
    for j in range(4):
        ve.tensor_tensor(ab[j][:], gj[j], ng[:, :, :, j : j + 1], op=amx)
    m1 = pool.tile(ash, f16, tag=pfx + "m1", name="m1")
    M1 = pool.tile(ash, f16, tag=pfx + "M1", name="M1")
    m2 = pool.tile(ash, f16, tag=pfx + "abs0", name="m2")
    M2 = pool.tile(ash, f16, tag=pfx + "abs1", name="M2")
    ve.tensor_tensor(m1[:], ab[0][:], ab[1][:], op=amin)
    ve.tensor_tensor(M1[:], ab[0][:], ab[1][:], op=amx)
    ve.tensor_tensor(m2[:], ab[2][:], ab[3][:], op=amin)
    ve.tensor_tensor(M2[:], ab[2][:], ab[3][:], op=amx)
    # 2nd smallest of the 4 = min(max(m1, m2), min(M1, M2))
    t = pool.tile(ash, f16, tag=pfx + "abs2", name="t")
    ve.tensor_tensor(m1[:], m1[:], m2[:], op=amx)
    ve.tensor_tensor(M1[:], M1[:], M2[:], op=amin)
    ve.tensor_tensor(t[:], m1[:], M1[:], op=amin)
    # t4: threshold broadcast over the group-of-4 axis
    t4 = pool.tile([P, nb, RANK], f16, tag=pfx + "t4", name="t4")
    h4 = t4[:].rearrange("p b (g q) -> p b g q", q=4)
    for j in range(4):
        ve.tensor_copy(h4[:, :, :, j : j + 1], t[:])
    # s = w - clip(w, -t, t)
    th = thr[:]
    nt4 = pool.tile([P, nb, RANK], f16, tag=pfx + "wneg", name="nt4")
    ve.tensor_scalar_mul(nt4[:], t4[:], -1.0)
    ve.tensor_tensor(th, wfh, t4[:], op=amin)
    ve.tensor_tensor(th, th, nt4[:], op=amx)
    ve.tensor_sub(th, wfh, th)
    if scale != 1.0:
        ve.tensor_scalar_mul(th, th, float(scale))
    if out_f16 is not None:
        ve.tensor_copy(out_f16[:], thr[:])
    return thr


def _build(scale_a, scale_b):
    import concourse.mybir as mybir
    import concourse.tile as tile
    from concourse import bacc
    from concourse.bass import ts
    from concourse.masks import make_identity

    f32, f16 = mybir.dt.float32, mybir.dt.float16

    nc = bacc.Bacc("TRN2", target_bir_lowering=False, debug=False,
                   enable_asserts=False)
    x_d = nc.dram_tensor("x", [IN_F, T_CORE], f16, kind="ExternalInput")
    wa_d = nc.dram_tensor("weight_A", [IN_F, RANK], f16, kind="ExternalInput")
    wb_d = nc.dram_tensor("weight_B", [OUT_F, RANK], f16, kind="ExternalInput")
    b_d = nc.dram_tensor("bias", [1, OUT_F], f16, kind="ExternalInput")
    o_d = nc.dram_tensor("out", [T_CORE, OUT_F], f16, kind="ExternalOutput")

    with tile.TileContext(nc) as tc:
        CH = 2                       # token chunks (pipelined load/store)
        CTOK = T_CORE // CH          # 1024 tokens per chunk
        CTB = CTOK // TBLK           # 2 mm1 psum blocks per chunk
        CTT = CTOK // P              # 8 mm2 token tiles per chunk
        with (
            tc.tile_pool(name="const", bufs=1) as constp,
            tc.tile_pool(name="wtmp", bufs=1) as wtmp,
            tc.tile_pool(name="bulk", bufs=N_K + 2) as bulkp,
            tc.tile_pool(name="outp", bufs=6) as outp,
            tc.tile_pool(name="proj", bufs=1) as projp,
            tc.tile_pool(name="ps1", bufs=1, space="PSUM") as ps1p,
            tc.tile_pool(name="ps2", bufs=6, space="PSUM") as ps2p,
        ):
            ident16 = constp.tile([P, P], f16)
            make_identity(nc, ident16[:])

            # --- x loads on the SP queue: per chunk, 32 k-tiles [128, 1024]
            # (2KB rows).  Chunk B's loads reuse chunk A's buffers (mm1 A
            # frees them ~1 us behind the load stream).
            # 2KB-row tiles run a single DGE queue at ~272 GB/s (descriptor
            # issue bound) -- alternate k across the SP and ACT queues.
            xts = [[None] * N_K for _ in range(CH)]
            for c in range(CH):
                for k in range(N_K):
                    bt = bulkp.tile([P, CTOK], f16, name="bulk", tag="bulk")
                    eng = nc.sync if k % 2 == 0 else nc.scalar
                    eng.dma_start(bt[:], x_d[ts(k, P), ts(c, CTOK)])
                    xts[c][k] = bt

            # --- weight prep; wA on DVE (mm1 needs it first), wB on Pool ---
            # wa16[c, k, r] = wA[k*128+c, r]  (host-permuted rows)
            wa16 = constp.tile([P, N_K, RANK], f16)
            _soft_threshold_weights(nc, wtmp, wa_d, scale_a, wa16,
                                    nc.vector, "a")
            wbt = constp.tile([RANK + 1, OUT_F], f16)  # wB.T (+ bias row)
            thr_b = _soft_threshold_weights(nc, wtmp, wb_d, scale_b, None,
                                            nc.vector, "b")
            nc.scalar.dma_start(wbt[RANK : RANK + 1, :], b_d[:])

            def wbt_quanta():
                """wbt transposes, interleaved into mm1 A's load-paced
                slack."""
                for b in range(OUT_F // P):
                    pw = ps2p.tile([P, MM2_N], f32, tag="ps2", name="pw")
                    pwv = pw[0:RANK, 0 : P // 2].bitcast(f16)
                    nc.tensor.transpose(pwv, thr_b[:, b, :], ident16[:])
                    if b % 2 == 0:
                        nc.scalar.copy(wbt[0:RANK, ts(b, P)], pwv)
                    else:
                        nc.vector.tensor_copy(wbt[0:RANK, ts(b, P)], pwv)
                    yield

            wq = wbt_quanta()

            def drain(it, n=None):
                if it is None:
                    return None
                try:
                    if n is None:
                        while True:
                            next(it)
                    else:
                        for _ in range(n):
                            next(it)
                except StopIteration:
                    return None
                return it

            # x_proj f16 staging (+ones row for the bias trick); chunk B's
            # cast lands in the right half after mm1 B.
            xpa = projp.tile([RANK + 1, T_CORE], f16)
            nc.gpsimd.memset(xpa[RANK : RANK + 1, :], 1.0)

            # PE FIFO: mm1 A -> mm2 A -> mm1 B -> mm2 B.  Chunk B reuses
            # chunk A's acc banks (freed by the cast), so ps2 gets 6 bufs.
            for c in range(CH):
                # --- mm1: k-outer so the PE consumes tiles as they land ---
                accs = [ps1p.tile([RANK, TBLK], f32, tag=f"acc{tb}",
                                  name=f"acc{tb}") for tb in range(CTB)]
                for k in range(N_K):
                    for tb in range(CTB):
                        nc.tensor.matmul(accs[tb][:], wa16[:, k, :],
                                         xts[c][k][:, ts(tb, TBLK)],
                                         start=(k == 0), stop=(k == N_K - 1))
                    if c == 0 and 8 <= k < 24:
                        wq = drain(wq, 2)
                wq = drain(wq)

                # --- x_proj cast to f16 ---
                nc.scalar.copy(xpa[0:RANK, ts(2 * c, TBLK)], accs[0][:])
                nc.vector.tensor_copy(xpa[0:RANK, ts(2 * c + 1, TBLK)],
                                      accs[1][:])

                # --- mm2 + PSUM drains (ACT/DVE) + stores (Pool queue) ---
                for t in range(CTT):
                    tt = c * CTT + t
                    ob = outp.tile([P, OUT_F], f16, name="ob", tag="ob")
                    for j in range(N_OB):
                        ps2 = ps2p.tile([P, MM2_N], f32, tag="ps2",
                                        name="ps2")
                        nc.tensor.matmul(ps2[:], xpa[:, ts(tt, P)],
                                         wbt[:, ts(j, MM2_N)],
                                         start=True, stop=True)
                        dst = ob[:, ts(j, MM2_N)]
                        if j % 2 == 0:
                            nc.vector.tensor_copy(dst, ps2[:])
                        else:
                            nc.scalar.copy(dst, ps2[:])
                    # chunk A stores must dodge the SP/ACT load queues;
                    # chunk B can split onto SP (its loads are done)
                    if c == 1 and t % 2 == 1:
                        nc.sync.dma_start(o_d[ts(tt, P), :], ob[:])
                    else:
                        nc.gpsimd.dma_start(o_d[ts(tt, P), :], ob[:])

    nc.compile()
    return nc


def get_nc(scale_a, scale_b):
    key = (float(scale_a), float(scale_b))
    if key not in _CACHE:
        _CACHE[key] = _build(*key)
    return _CACHE[key]


def make_in_maps(x, weight_A, weight_B, bias):
    """Host-side shard + f16 cast + transpose: per-core input dicts."""
    x16 = np.asarray(x, dtype=np.float32).astype(np.float16)
    wa = np.asarray(weight_A, np.float32).astype(np.float16)
    wb = np.asarray(weight_B, np.float32).astype(np.float16)
    # Lossless row permutations so the device DMA is contiguous (4KB
    # descriptors instead of 128B row gathers):
    #   wa16[c, k, r] = wA[k*128+c, r] -> send rows in (c,k) order
    #   thr_b[p, b, r] = wB[b*128+p, r] -> send rows in (p,b) order
    wa = np.ascontiguousarray(
        wa.reshape(N_K, P, RANK).transpose(1, 0, 2).reshape(IN_F, RANK))
    wb = np.ascontiguousarray(
        wb.reshape(OUT_F // P, P, RANK).transpose(1, 0, 2)
        .reshape(OUT_F, RANK))
    bi = np.ascontiguousarray(
        np.asarray(bias, np.float32).astype(np.float16)).reshape(1, OUT_F)
    xf = x16.reshape(N_TOK, IN_F)
    return [
        {
            # transposed shard: [IN_F, T_CORE] f16
            "x": np.ascontiguousarray(
                xf[c * T_CORE : (c + 1) * T_CORE].T),
            "weight_A": wa,
            "weight_B": wb,
            "bias": bi,
        }
        for c in range(N_CORES)
    ]


def kernel(x, weight_A, weight_B, bias, scale_A, scale_B):
    from concourse.bass_utils import run_bass_kernel_spmd

    sa = float(np.asarray(scale_A))
    sb = float(np.asarray(scale_B))
    nc = get_nc(sa, sb)

    in_maps = make_in_maps(x, weight_A, weight_B, bias)
    res = run_bass_kernel_spmd(nc, in_maps, core_ids=list(range(N_CORES)))
    out = np.concatenate([r["out"] for r in res.results], axis=0)
    return out.astype(np.float32).reshape(B, S, OUT_F)


# revision 24
# speedup vs baseline: 1.1383x; 1.1383x over previous
"""Trainium2 Bass kernel: low-rank (LoRA-style) linear with 2:4 soft-threshold
pruned weights, fp16 matmul / fp32 accumulate.

  wA = soft_threshold24(weight_A) * scale_A          # [IN, R]
  wB = soft_threshold24(weight_B) * scale_B          # [OUT, R]
  x_proj = f16(x) @ f16(wA)            (f32 accum)   # [N, R]
  out    = f16(x_proj) @ f16(wB).T + bias            # [N, OUT]

Sharding: data-parallel over the token dim across 8 cores (2048 tokens/core),
small weights replicated. No collectives.

v2 layout: the host uploads each core's x shard already transposed
([IN_F, T_CORE] f16), so mm1 is a plain f16 matmul (wA tile stationary,
xT tile moving) with NO on-device transposes of x -- the v1 kernel spent
half its PE time on f32 pair-transposes + PSUM drain copies. PE work drops
to 256 matmuls of 512 cols; the kernel becomes DMA-bound (~33.5 MB at
~360 GB/s => ~96 us floor).

Schedule: 32 xT k-tiles [128, 2048] stream in on the SP queue (0.5 MB
contiguous each). mm1 runs k-outer / token-block-inner, accumulating
x_proj[64, 2048] across 4 PSUM banks, so it consumes tiles as they land.
wA's threshold runs on DVE (needed by mm1 k=0 early); wB's runs on the
Pool engine concurrently (only needed ~45 us in). The 32 wB.T PE
transposes hide in mm1's tail slack (k>=24, load-paced). mm2 then streams
16 token tiles x 8 out-blocks vs the wbt [65, 4096] stationary-side tiles
(ones row + bias row fold the bias into the matmul), drains PSUM on
alternating ACT/DVE, and stores ride the now-idle SP queue.
"""

import sys

import numpy as np

if "/opt/trn_rl_repo" not in sys.path:
    sys.path.insert(0, "/opt/trn_rl_repo")

B, S, IN_F, OUT_F, RANK = 4, 4096, 4096, 4096, 64
N_CORES = 8
N_TOK = B * S                   # 16384
T_CORE = N_TOK // N_CORES       # 2048 tokens per core
P = 128
N_K = IN_F // P                 # 32 contraction tiles
TB = 4                          # token blocks for mm1 PSUM banks
TBLK = T_CORE // TB             # 512
MM2_N = 512
N_OB = OUT_F // MM2_N           # 8 output column groups
N_TT = T_CORE // P              # 16 token tiles for mm2

_CACHE = {}


def _soft_threshold_weights(nc, pool, wfh, nb, scale, out_ap, eng, pfx):
    """Emit IR computing soft_threshold24(wfh) * scale (f16) into out_ap.

    wfh: [P, nb, RANK] f16 AP of staged weights (host-permuted rows; the
    2:4 threshold is elementwise over groups of 4 along R, so any row
    permutation works).
    """
    import concourse.mybir as mybir

    f16 = mybir.dt.float16
    amin = mybir.AluOpType.min
    amx = mybir.AluOpType.max
    ve = eng
    g = wfh.rearrange("p b (g q) -> p b g q", q=4)
    gj = [g[:, :, :, j : j + 1] for j in range(4)]
    ash = [P, nb, RANK // 4, 1]
    wneg = pool.tile([P, nb, RANK], f16, tag=pfx + "wneg", name="wneg")
    ve.tensor_scalar_mul(wneg[:], wfh, -1.0)
    ng = wneg[:].rearrange("p b (g q) -> p b g q", q=4)
    ab = [pool.tile(ash, f16, tag=f"{pfx}abs{j}", name=f"abs{j}")
          for j in range(4)]
    for j in range(4):
        ve.tensor_tensor(ab[j][:], gj[j], ng[:, :, :, j : j + 1], op=amx)
    m1 = pool.tile(ash, f16, tag=pfx + "m1", name="m1")
    M1 = pool.tile(ash, f16, tag=pfx + "M1", name="M1")
    m2 = pool.tile(ash, f16, tag=pfx + "abs0", name="m2")
    M2 = pool.tile(ash, f16, tag=pfx + "abs1", name="M2")
    ve.tensor_tensor(m1[:], ab[0][:], ab[1][:], op=amin)
    ve.tensor_tensor(M1[:], ab[0][:], ab[1][:], op=amx)
    ve.tensor_tensor(m2[:], ab[2][:], ab[3][:], op=amin)
    ve.tensor_tensor(M2[:], ab[2][:], ab[3][:], op=amx)
    # 2nd smallest of the 4 = min(max(m1, m2), min(M1, M2))
    t = pool.tile(ash, f16, tag=pfx + "abs2", name="t")
    ve.tensor_tensor(m1[:], m1[:], m2[:], op=amx)
    ve.tensor_tensor(M1[:], M1[:], M2[:], op=amin)
    ve.tensor_tensor(t[:], m1[:], M1[:], op=amin)
    # t4: threshold broadcast over the group-of-4 axis
    t4 = pool.tile([P, nb, RANK], f16, tag=pfx + "t4", name="t4")
    h4 = t4[:].rearrange("p b (g q) -> p b g q", q=4)
    for j in range(4):
        ve.tensor_copy(h4[:, :, :, j : j + 1], t[:])
    # s = w - clip(w, -t, t), written directly into out_ap
    th = out_ap
    nt4 = pool.tile([P, nb, RANK], f16, tag=pfx + "wneg", name="nt4")
    ve.tensor_scalar_mul(nt4[:], t4[:], -1.0)
    ve.tensor_tensor(th, wfh, t4[:], op=amin)
    ve.tensor_tensor(th, th, nt4[:], op=amx)
    ve.tensor_sub(th, wfh, th)
    if scale != 1.0:
        ve.tensor_scalar_mul(th, th, float(scale))


def _build(scale_a, scale_b):
    import concourse.mybir as mybir
    import concourse.tile as tile
    from concourse import bacc
    from concourse.bass import ts
    from concourse.masks import make_identity

    f32, f16 = mybir.dt.float32, mybir.dt.float16

    nc = bacc.Bacc("TRN2", target_bir_lowering=False, debug=False,
                   enable_asserts=False)
    # packed transposed x: row (c*8+g)*128+p, col kk*1024+t
    #   = f16(x)[core, c*1024+t, (4g+kk)*128+p]
    x_d = nc.dram_tensor("x", [T_CORE, IN_F], f16, kind="ExternalInput")
    wa_d = nc.dram_tensor("weight_A", [IN_F, RANK], f16, kind="ExternalInput")
    wb_d = nc.dram_tensor("weight_B", [OUT_F, RANK], f16, kind="ExternalInput")
    b_d = nc.dram_tensor("bias", [1, OUT_F], f16, kind="ExternalInput")
    o_d = nc.dram_tensor("out", [T_CORE, OUT_F], f16, kind="ExternalOutput")

    with tile.TileContext(nc) as tc:
        CH = 2                       # token chunks (pipelined load/store)
        CTOK = T_CORE // CH          # 1024 tokens per chunk
        CTB = CTOK // TBLK           # 2 mm1 psum blocks per chunk
        CTT = CTOK // P              # 8 mm2 token tiles per chunk
        with (
            tc.tile_pool(name="const", bufs=1) as constp,
            tc.tile_pool(name="wtmp", bufs=1) as wtmp,
            tc.tile_pool(name="bulk", bufs=10) as bulkp,
            tc.tile_pool(name="outp", bufs=6) as outp,
            tc.tile_pool(name="proj", bufs=1) as projp,
            tc.tile_pool(name="ps1", bufs=1, space="PSUM") as ps1p,
            tc.tile_pool(name="ps2", bufs=6, space="PSUM") as ps2p,
        ):
            ident16 = constp.tile([P, P], f16)
            make_identity(nc, ident16[:])

            # --- weight staging DMAs FIRST on the SP queue (land ~10 us;
            # the DVE threshold chain gates mm1's start) ---
            NB_B = OUT_F // P
            wfa = wtmp.tile([P, N_K, RANK], f16, tag="awstage", name="wfa")
            nc.sync.dma_start(wfa[:],
                              wa_d[:].rearrange("(c n) r -> c n r", c=P))
            wfb = wtmp.tile([P, NB_B, RANK], f16, tag="bwstage", name="wfb")
            nc.sync.dma_start(wfb[:],
                              wb_d[:].rearrange("(c n) r -> c n r", c=P))

            # --- x loads on the SP queue.  Tiles are [128, 4096] f16 = 8KB
            # partition rows (2KB rows run ~270 GB/s descriptor-bound; 8KB
            # run at full rate).  The host packs 4 k-rows x 1024 chunk-tokens
            # per partition row so chunking keeps full-size descriptors.
            # Chunk B's loads reuse chunk A's buffers.
            KQ = 4
            xts = [[None] * (N_K // KQ) for _ in range(CH)]
            for c in range(CH):
                for g in range(N_K // KQ):
                    bt = bulkp.tile([P, KQ * CTOK], f16, name="bulk",
                                    tag="bulk")
                    nc.sync.dma_start(
                        bt[:], x_d[ts(c * (N_K // KQ) + g, P), :])
                    xts[c][g] = bt

            # --- weight prep on DVE; wA first (mm1 needs it), in TWO half
            # tiles so mm1 k=0..15 starts after half the chain ---
            HK = N_K // 2
            wa16h = [constp.tile([P, HK, RANK], f16, tag=f"wa16h{h}",
                                 name=f"wa16h{h}") for h in range(2)]
            for h in range(2):
                _soft_threshold_weights(
                    nc, wtmp, wfa[:, h * HK : (h + 1) * HK, :], HK, scale_a,
                    wa16h[h][:], nc.vector, "a")
            wbt = constp.tile([RANK + 1, OUT_F], f16)  # wB.T (+ bias row)
            thr_b = wtmp.tile([P, NB_B, RANK], f16, tag="bthr", name="bthr")
            _soft_threshold_weights(nc, wtmp, wfb[:], NB_B, scale_b,
                                    thr_b[:], nc.vector, "b")
            nc.scalar.dma_start(wbt[RANK : RANK + 1, :], b_d[:])

            def wbt_quanta():
                """wbt transposes, interleaved into mm1 A's load-paced
                slack."""
                for b in range(OUT_F // P):
                    pw = ps2p.tile([P, MM2_N], f32, tag="ps2", name="pw")
                    pwv = pw[0:RANK, 0 : P // 2].bitcast(f16)
                    nc.tensor.transpose(pwv, thr_b[:, b, :], ident16[:])
                    if b % 2 == 0:
                        nc.scalar.copy(wbt[0:RANK, ts(b, P)], pwv)
                    else:
                        nc.vector.tensor_copy(wbt[0:RANK, ts(b, P)], pwv)
                    yield

            wq = wbt_quanta()

            def drain(it, n=None):
                if it is None:
                    return None
                try:
                    if n is None:
                        while True:
                            next(it)
                    else:
                        for _ in range(n):
                            next(it)
                except StopIteration:
                    return None
                return it

            # x_proj f16 staging (+ones row for the bias trick); chunk B's
            # cast lands in the right half after mm1 B.
            xpa = projp.tile([RANK + 1, T_CORE], f16)
            nc.gpsimd.memset(xpa[RANK : RANK + 1, :], 1.0)

            def mm1_quanta(c, accs):
                """Per-k mm1 quanta for chunk c (CTB matmuls each)."""
                for k in range(N_K):
                    g, kk = k // KQ, k % KQ
                    for tb in range(CTB):
                        nc.tensor.matmul(
                            accs[tb][:], wa16h[k // HK][:, k % HK, :],
                            xts[c][g][:, kk * CTOK + tb * TBLK
                                      : kk * CTOK + (tb + 1) * TBLK],
                            start=(k == 0), stop=(k == N_K - 1))
                    yield

            def cast_xproj(c, accs):
                nc.scalar.copy(xpa[0:RANK, ts(2 * c, TBLK)], accs[0][:])
                nc.vector.tensor_copy(xpa[0:RANK, ts(2 * c + 1, TBLK)],
                                      accs[1][:])

            def mm2_chunk(c, fill):
                """mm2 + 3-way PSUM drains + stores; `fill` quanta (chunk
                B's mm1) absorb the PE's drain-stall slots."""
                for t in range(CTT):
                    tt = c * CTT + t
                    ob = outp.tile([P, OUT_F], f16, name="ob", tag="ob")
                    for j in range(N_OB):
                        ps2 = ps2p.tile([P, MM2_N], f32, tag="ps2",
                                        name="ps2")
                        nc.tensor.matmul(ps2[:], xpa[:, ts(tt, P)],
                                         wbt[:, ts(j, MM2_N)],
                                         start=True, stop=True)
                        dst = ob[:, ts(j, MM2_N)]
                        if j % 2 == 0:
                            nc.vector.tensor_copy(dst, ps2[:])
                        else:
                            nc.scalar.copy(dst, ps2[:])
                        # 1 fill quantum per 2 matmuls: chunk B's last
                        # tiles land only ~2/3 into mm2 A
                        if j % 2 == 0:
                            fill = drain(fill, 1)
                    # chunk A stores dodge the SP load queue; chunk B
                    # splits across the Pool and SP queues
                    if c == 1 and t % 2 == 1:
                        nc.sync.dma_start(o_d[ts(tt, P), :], ob[:])
                    else:
                        nc.gpsimd.dma_start(o_d[ts(tt, P), :], ob[:])
                return fill

            # PE FIFO: mm1 A -> [mm2 A | mm1 B interleaved] -> mm2 B.
            # Chunk B reuses chunk A's acc banks, so ps2 gets 6 bufs.
            accsA = [ps1p.tile([RANK, TBLK], f32, tag=f"acc{tb}",
                               name=f"acc{tb}") for tb in range(CTB)]
            q = mm1_quanta(0, accsA)
            for k in range(N_K):
                q = drain(q, 1)
                if 8 <= k < 24:
                    wq = drain(wq, 2)
            wq = drain(wq)
            cast_xproj(0, accsA)

            accsB = [ps1p.tile([RANK, TBLK], f32, tag=f"acc{tb}",
                               name=f"acc{tb}") for tb in range(CTB)]
            m1b = mm1_quanta(1, accsB)
            m1b = mm2_chunk(0, m1b)
            drain(m1b)
            cast_xproj(1, accsB)
            mm2_chunk(1, None)

    nc.compile()
    return nc


def get_nc(scale_a, scale_b):
    key = (float(scale_a), float(scale_b))
    if key not in _CACHE:
        _CACHE[key] = _build(*key)
    return _CACHE[key]


def make_in_maps(x, weight_A, weight_B, bias):
    """Host-side shard + f16 cast + transpose: per-core input dicts."""
    x16 = np.asarray(x, dtype=np.float32).astype(np.float16)
    wa = np.asarray(weight_A, np.float32).astype(np.float16)
    wb = np.asarray(weight_B, np.float32).astype(np.float16)
    # Lossless row permutations so the device DMA is contiguous (4KB
    # descriptors instead of 128B row gathers):
    #   wa16[c, k, r] = wA[k*128+c, r] -> send rows in (c,k) order
    #   thr_b[p, b, r] = wB[b*128+p, r] -> send rows in (p,b) order
    wa = np.ascontiguousarray(
        wa.reshape(N_K, P, RANK).transpose(1, 0, 2).reshape(IN_F, RANK))
    wb = np.ascontiguousarray(
        wb.reshape(OUT_F // P, P, RANK).transpose(1, 0, 2)
        .reshape(OUT_F, RANK))
    bi = np.ascontiguousarray(
        np.asarray(bias, np.float32).astype(np.float16)).reshape(1, OUT_F)
    xf = x16.reshape(N_TOK, IN_F)

    def pack_x(core):
        # [IN, T] transposed shard, packed so each [128, 4096] device tile
        # holds 4 k-rows x 1024 chunk-tokens per partition (8KB DMA rows):
        #   x_d[(c*8+g)*128+p, kk*1024+t] = xT[(4g+kk)*128+p, c*1024+t]
        xt = xf[core * T_CORE : (core + 1) * T_CORE].T  # [4096, 2048]
        a = xt.reshape(IN_F // 512, 4, P, 2, 1024)      # [g, kk, p, c, t]
        a = a.transpose(3, 0, 2, 1, 4)                  # [c, g, p, kk, t]
        return np.ascontiguousarray(a.reshape(T_CORE, IN_F))

    return [
        {
            "x": pack_x(c),
            "weight_A": wa,
            "weight_B": wb,
            "bias": bi,
        }
        for c in range(N_CORES)
    ]


def kernel(x, weight_A, weight_B, bias, scale_A, scale_B):
    from concourse.bass_utils import run_bass_kernel_spmd

    sa = float(np.asarray(scale_A))
    sb = float(np.asarray(scale_B))
    nc = get_nc(sa, sb)

    in_maps = make_in_maps(x, weight_A, weight_B, bias)
    res = run_bass_kernel_spmd(nc, in_maps, core_ids=list(range(N_CORES)))
    out = np.concatenate([r["out"] for r in res.results], axis=0)
    return out.astype(np.float32).reshape(B, S, OUT_F)


# revision 26
# speedup vs baseline: 1.2041x; 1.0579x over previous
"""Trainium2 Bass kernel: low-rank (LoRA-style) linear with 2:4 soft-threshold
pruned weights, fp16 matmul / fp32 accumulate.

  wA = soft_threshold24(weight_A) * scale_A          # [IN, R]
  wB = soft_threshold24(weight_B) * scale_B          # [OUT, R]
  x_proj = f16(x) @ f16(wA)            (f32 accum)   # [N, R]
  out    = f16(x_proj) @ f16(wB).T + bias            # [N, OUT]

Sharding: data-parallel over the token dim across 8 cores (2048 tokens/core),
small weights replicated. No collectives.

Host does dtype/layout prep only (f16 cast, transpose, row packing); all of
the module's math (threshold, both matmuls, bias) runs on device.

Structure (v8): the host uploads each core's x shard transposed and packed
so every DMA tile is [128, 4096] f16 (8KB partition rows = full ~420 GB/s;
narrow rows are descriptor-bound ~270).  Tokens are processed in 4 chunks of
512; the PE FIFO runs mm1(A) -> transposes(wB.T) -> [mm2(c) with mm1(c+1)
matmuls woven into the drain-stall slots] -> ... -> mm2(D).  Chunk stores
overlap later chunks' compute on the Pool/SP DGE queues.  PSUM: 2 banks
ping-pong the x_proj accumulators, 6 banks give mm2 three [128,1024]
pair-drain buffers (one ACT/DVE copy moves two matmuls' output).  The 2:4
soft-threshold runs on DVE with abs_max + per-group clips (no full-width
temporaries); wA/wB are each done in halves so consumers unblock early.
"""

import sys

import numpy as np

if "/opt/trn_rl_repo" not in sys.path:
    sys.path.insert(0, "/opt/trn_rl_repo")

B, S, IN_F, OUT_F, RANK = 4, 4096, 4096, 4096, 64
N_CORES = 8
N_TOK = B * S                   # 16384
T_CORE = N_TOK // N_CORES       # 2048 tokens per core
P = 128
N_K = IN_F // P                 # 32 contraction tiles
MM2_N = 512
N_OB = OUT_F // MM2_N           # 8 output column groups

CH = 4                          # token chunks (pipelined)
CTOK = T_CORE // CH             # 512 tokens per chunk
KQ = 8                          # k-rows packed per load tile
NG = N_K // KQ                  # 4 load tiles per chunk
CTT = CTOK // P                 # 4 mm2 token tiles per chunk

_CACHE = {}


def _soft_threshold(nc, pool, wfh, nb, scale, out_ap, pfx):
    """soft_threshold24(wfh)*scale -> out_ap (f16), on DVE.

    wfh: [P, nb, RANK] f16 AP (host-permuted rows; the threshold is
    elementwise over groups of 4 along R, so row order is free).
    t = 2nd-smallest |.| of each group of 4; out = w - clip(w, -t, t).
    All ops are group-strided (RANK/4 wide) -- no full-width temporaries.
    """
    import concourse.mybir as mybir

    f16 = mybir.dt.float16
    amin, amx = mybir.AluOpType.min, mybir.AluOpType.max
    ve = nc.vector

    g = wfh.rearrange("p b (g q) -> p b g q", q=4)
    og = out_ap.rearrange("p b (g q) -> p b g q", q=4)
    gj = [g[:, :, :, j : j + 1] for j in range(4)]
    ash = [P, nb, RANK // 4, 1]
    wneg = pool.tile([P, nb, RANK], f16, tag=pfx + "wneg", name="wneg")
    ve.tensor_scalar_mul(wneg[:], wfh, -1.0)
    ng = wneg[:].rearrange("p b (g q) -> p b g q", q=4)
    ab = [pool.tile(ash, f16, tag=f"{pfx}ab{j}", name=f"ab{j}")
          for j in range(4)]
    for j in range(4):
        ve.tensor_tensor(ab[j][:], gj[j], ng[:, :, :, j : j + 1], op=amx)
    m1 = pool.tile(ash, f16, tag=pfx + "m1", name="m1")
    M1 = pool.tile(ash, f16, tag=pfx + "M1", name="M1")
    m2 = pool.tile(ash, f16, tag=pfx + "m2", name="m2")
    M2 = pool.tile(ash, f16, tag=pfx + "M2", name="M2")
    ve.tensor_tensor(m1[:], ab[0][:], ab[1][:], op=amin)
    ve.tensor_tensor(M1[:], ab[0][:], ab[1][:], op=amx)
    ve.tensor_tensor(m2[:], ab[2][:], ab[3][:], op=amin)
    ve.tensor_tensor(M2[:], ab[2][:], ab[3][:], op=amx)
    # t = 2nd smallest = min(max(m1, m2), min(M1, M2))
    t = pool.tile(ash, f16, tag=pfx + "t", name="t")
    ve.tensor_tensor(m1[:], m1[:], m2[:], op=amx)
    ve.tensor_tensor(M1[:], M1[:], M2[:], op=amin)
    ve.tensor_tensor(t[:], m1[:], M1[:], op=amin)
    nt = pool.tile(ash, f16, tag=pfx + "nt", name="nt")
    ve.tensor_scalar_mul(nt[:], t[:], -1.0)
    # out_j = g_j - clip(g_j, -t, t), per group lane (ab_j reused as tmp)
    for j in range(4):
        ve.tensor_tensor(ab[j][:], gj[j], t[:], op=amin)
        ve.tensor_tensor(ab[j][:], ab[j][:], nt[:], op=amx)
        ve.tensor_sub(og[:, :, :, j : j + 1], gj[j], ab[j][:])
    if scale != 1.0:
        ve.tensor_scalar_mul(out_ap, out_ap, float(scale))


def _build(scale_a, scale_b):
    import concourse.mybir as mybir
    import concourse.tile as tile
    from concourse import bacc
    from concourse.bass import ts
    from concourse.masks import make_identity

    f32, f16 = mybir.dt.float32, mybir.dt.float16

    nc = bacc.Bacc("TRN2", target_bir_lowering=False, debug=False,
                   enable_asserts=False)
    # packed transposed x: row (c*NG+g)*128+p, col kk*CTOK+t
    #   = f16(x)[core, c*CTOK+t, (KQ*g+kk)*128+p]
    x_d = nc.dram_tensor("x", [T_CORE, IN_F], f16, kind="ExternalInput")
    wa_d = nc.dram_tensor("weight_A", [IN_F, RANK], f16, kind="ExternalInput")
    wb_d = nc.dram_tensor("weight_B", [OUT_F, RANK], f16, kind="ExternalInput")
    b_d = nc.dram_tensor("bias", [1, OUT_F], f16, kind="ExternalInput")
    o_d = nc.dram_tensor("out", [T_CORE, OUT_F], f16, kind="ExternalOutput")

    with tile.TileContext(nc) as tc:
        with (
            tc.tile_pool(name="const", bufs=1) as constp,
            tc.tile_pool(name="wtmp", bufs=1) as wtmp,
            tc.tile_pool(name="bulk", bufs=8) as bulkp,
            tc.tile_pool(name="outp", bufs=6) as outp,
            tc.tile_pool(name="proj", bufs=1) as projp,
            tc.tile_pool(name="ps1", bufs=1, space="PSUM") as ps1p,
            tc.tile_pool(name="ps2", bufs=3, space="PSUM") as ps2p,
        ):
            ident16 = constp.tile([P, P], f16)
            make_identity(nc, ident16[:])

            # --- weight staging DMAs first on the SP queue (the DVE
            # threshold chain gates mm1/mm2 starts) ---
            NB_B = OUT_F // P
            wfa = wtmp.tile([P, N_K, RANK], f16, tag="awstage", name="wfa")
            nc.sync.dma_start(wfa[:],
                              wa_d[:].rearrange("(c n) r -> c n r", c=P))
            wfb = wtmp.tile([P, NB_B, RANK], f16, tag="bwstage", name="wfb")
            nc.sync.dma_start(wfb[:],
                              wb_d[:].rearrange("(c n) r -> c n r", c=P))

            # --- x loads on the SP queue: 16 tiles [128, 4096] f16 ---
            xts = [[None] * NG for _ in range(CH)]
            for c in range(CH):
                for g in range(NG):
                    bt = bulkp.tile([P, KQ * CTOK], f16, name="bulk",
                                    tag="bulk")
                    nc.sync.dma_start(bt[:], x_d[ts(c * NG + g, P), :])
                    xts[c][g] = bt

            # --- thresholds on DVE, each weight in two halves so
            # consumers unblock at half-chain latency; wA first ---
            HK = N_K // 2
            wa16h = [constp.tile([P, HK, RANK], f16, tag=f"wa16h{h}",
                                 name=f"wa16h{h}") for h in range(2)]
            for h in range(2):
                _soft_threshold(nc, wtmp, wfa[:, h * HK : (h + 1) * HK, :],
                                HK, scale_a, wa16h[h][:], "a")
            HB = NB_B // 2
            thr_bh = [wtmp.tile([P, HB, RANK], f16, tag=f"bthr{h}",
                                name=f"bthr{h}") for h in range(2)]
            for h in range(2):
                _soft_threshold(nc, wtmp, wfb[:, h * HB : (h + 1) * HB, :],
                                HB, scale_b, thr_bh[h][:], "b")
            wbt = constp.tile([RANK + 1, OUT_F], f16)  # wB.T (+ bias row)
            nc.scalar.dma_start(wbt[RANK : RANK + 1, :], b_d[:])

            # x_proj f16 staging (+ones row for the bias trick)
            xpa = projp.tile([RANK + 1, T_CORE], f16)
            nc.gpsimd.memset(xpa[RANK : RANK + 1, :], 1.0)

            def drain(it, n=None):
                if it is None:
                    return None
                try:
                    if n is None:
                        while True:
                            next(it)
                    else:
                        for _ in range(n):
                            next(it)
                except StopIteration:
                    return None
                return it

            def mm1_quanta(c, acc):
                """Per-k mm1 quanta for chunk c: acc[64, 512] += wa_k^T x."""
                for k in range(N_K):
                    g, kk = k // KQ, k % KQ
                    nc.tensor.matmul(acc[:], wa16h[k // HK][:, k % HK, :],
                                     xts[c][g][:, ts(kk, CTOK)],
                                     start=(k == 0), stop=(k == N_K - 1))
                    yield

            def cast_xproj(c, acc):
                if c % 2 == 0:
                    nc.scalar.copy(xpa[0:RANK, ts(c, CTOK)], acc[:])
                else:
                    nc.vector.tensor_copy(xpa[0:RANK, ts(c, CTOK)], acc[:])

            def mm2_chunk(c, fill):
                """mm2 for chunk c: [128,1024] PSUM pair-drains on ACT/DVE;
                `fill` (next chunk's mm1) absorbs PE drain-stall slots."""
                for t in range(CTT):
                    tt = c * CTT + t
                    ob = outp.tile([P, OUT_F], f16, name="ob", tag="ob")
                    for jp in range(N_OB // 2):
                        ps2 = ps2p.tile([P, 2 * MM2_N], f32, tag="ps2",
                                        name="ps2")
                        for h in range(2):
                            nc.tensor.matmul(ps2[:, ts(h, MM2_N)],
                                             xpa[:, ts(tt, P)],
                                             wbt[:, ts(2 * jp + h, MM2_N)],
                                             start=True, stop=True)
                            fill = drain(fill, 1)
                        dst = ob[:, ts(jp, 2 * MM2_N)]
                        if jp % 2 == 0:
                            nc.vector.tensor_copy(dst, ps2[:])
                        else:
                            nc.scalar.copy(dst, ps2[:])
                    # stores: Pool DGE queue; last chunk splits onto SP
                    # (its loads are long done)
                    if c == CH - 1 and t % 2 == 1:
                        nc.sync.dma_start(o_d[ts(tt, P), :], ob[:])
                    else:
                        nc.gpsimd.dma_start(o_d[ts(tt, P), :], ob[:])
                return fill

            # --- PE FIFO: mm1(A); wbt transposes; then per chunk:
            # mm2(c) with mm1(c+1) woven in ---
            accs = [None] * CH
            accs[0] = ps1p.tile([RANK, CTOK], f32, tag="acc0", name="acc0")
            drain(mm1_quanta(0, accs[0]))

            for b in range(NB_B):
                pw = ps2p.tile([P, 2 * MM2_N], f32, tag="ps2", name="pw")
                pwv = pw[0:RANK, 0 : P // 2].bitcast(f16)
                nc.tensor.transpose(pwv, thr_bh[b // HB][:, b % HB, :],
                                    ident16[:])
                if b % 2 == 0:
                    nc.scalar.copy(wbt[0:RANK, ts(b, P)], pwv)
                else:
                    nc.vector.tensor_copy(wbt[0:RANK, ts(b, P)], pwv)
            cast_xproj(0, accs[0])

            fill = None
            for c in range(CH):
                if c + 1 < CH:
                    accs[c + 1] = ps1p.tile([RANK, CTOK], f32,
                                            tag=f"acc{(c + 1) % 2}",
                                            name="accn")
                    fill = mm1_quanta(c + 1, accs[c + 1])
                else:
                    fill = None
                fill = mm2_chunk(c, fill)
                drain(fill)
                if c + 1 < CH:
                    cast_xproj(c + 1, accs[c + 1])

    nc.compile()
    return nc


def get_nc(scale_a, scale_b):
    key = (float(scale_a), float(scale_b))
    if key not in _CACHE:
        _CACHE[key] = _build(*key)
    return _CACHE[key]


def make_in_maps(x, weight_A, weight_B, bias):
    """Host-side shard + f16 cast + transpose/pack: per-core input dicts."""
    x16 = np.asarray(x, dtype=np.float32).astype(np.float16)
    wa = np.asarray(weight_A, np.float32).astype(np.float16)
    wb = np.asarray(weight_B, np.float32).astype(np.float16)
    # Lossless row permutations so the device DMA is contiguous (4KB
    # descriptors instead of 128B row gathers):
    #   wa16[c, k, r] = wA[k*128+c, r] -> send rows in (c,k) order
    #   thr_b[p, b, r] = wB[b*128+p, r] -> send rows in (p,b) order
    wa = np.ascontiguousarray(
        wa.reshape(N_K, P, RANK).transpose(1, 0, 2).reshape(IN_F, RANK))
    wb = np.ascontiguousarray(
        wb.reshape(OUT_F // P, P, RANK).transpose(1, 0, 2)
        .reshape(OUT_F, RANK))
    bi = np.ascontiguousarray(
        np.asarray(bias, np.float32).astype(np.float16)).reshape(1, OUT_F)
    xf = x16.reshape(N_TOK, IN_F)

    def pack_x(core):
        # [IN, T] transposed shard, packed so each [128, 4096] device tile
        # holds KQ k-rows x CTOK chunk-tokens per partition (8KB DMA rows):
        #   x_d[(c*NG+g)*128+p, kk*CTOK+t] = xT[(KQ*g+kk)*128+p, c*CTOK+t]
        xt = xf[core * T_CORE : (core + 1) * T_CORE].T  # [IN_F, T_CORE]
        a = xt.reshape(NG, KQ, P, CH, CTOK)             # [g, kk, p, c, t]
        a = a.transpose(3, 0, 2, 1, 4)                  # [c, g, p, kk, t]
        return np.ascontiguousarray(a.reshape(T_CORE, IN_F))

    return [
        {
            "x": pack_x(c),
            "weight_A": wa,
            "weight_B": wb,
            "bias": bi,
        }
        for c in range(N_CORES)
    ]


def kernel(x, weight_A, weight_B, bias, scale_A, scale_B):
    from concourse.bass_utils import run_bass_kernel_spmd

    sa = float(np.asarray(scale_A))
    sb = float(np.asarray(scale_B))
    nc = get_nc(sa, sb)

    in_maps = make_in_maps(x, weight_A, weight_B, bias)
    res = run_bass_kernel_spmd(nc, in_maps, core_ids=list(range(N_CORES)))
    out = np.concatenate([r["out"] for r in res.results], axis=0)
    return out.astype(np.float32).reshape(B, S, OUT_F)
